# revision 17
# baseline (speedup 1.0000x reference)
"""GAT 2-layer kernel for Trainium2, 8 NeuronCores.

Strategy: nodes are permuted into 784 degree-balanced dst-blocks of 128
slots (98 blocks per core). All per-edge feature movement happens ON
DEVICE via gpsimd.dma_gather from a node-feature table in device HBM;
the halo exchange is an on-device jax.lax.all_gather between Bass
phases. Only ~60MB (bf16 X shards + compact int16 edge indices) crosses
the host<->device tunnel, vs ~1.5GB for host-side edge staging.

Pipeline (5 device dispatches, intermediates stay on device):
  p1 : tab1 = Xs @ W1e            -> per-core [NLOC, 192] f32 table rows
       row = [feat_h0(64) | 1 | feat_h1(64) | 1 | as0 as1 ad0 ad1 | 0pad]
  ag1: all_gather -> g4 [NSLOT/4, 768] (4 slots per row; int16 gather idx)
  p2 : layer-1 edge aggregation (dma_gather + one-hot-matmul scatter in
       PSUM) + relu + dense-2 -> tab2 [NLOC, 128] f32
       row = [feat(64) | 1 | as2 | ad2 | 0pad]
  ag2: all_gather -> g24 [NSLOT/4, 512]
  p3 : layer-2 edge aggregation -> out [NLOC, 64] bf16
"""
import time
from functools import partial

import numpy as np
import ml_dtypes

import jax

try:  # persistent XLA compile cache (saves ~8s/process on warm runs)
    jax.config.update("jax_compilation_cache_dir", "/tmp/gat_jax_cache")
    jax.config.update("jax_persistent_cache_min_compile_time_secs", 0.0)
    jax.config.update("jax_persistent_cache_min_entry_size_bytes", 0)
except Exception:
    pass

import jax.numpy as jnp
from jax.sharding import Mesh, NamedSharding, PartitionSpec as P
from jax.experimental.shard_map import shard_map

import concourse.bacc as bacc
import concourse.bass as bass
import concourse.mybir as mybir
import concourse.tile as tile
from concourse.bass2jax import bass_jit
from concourse.library_config import mlp

F32 = mybir.dt.float32
BF16 = mybir.dt.bfloat16
I16 = mybir.dt.int16
I32 = mybir.dt.int32
U8 = mybir.dt.uint8
AF = mybir.ActivationFunctionType
OP = mybir.AluOpType

PT = 128
NCORE = 8
NEG = 0.2

LAST_WALL = {}
LAST_EXEC_NS = {}
DBG = {}


def _host_prep(X, E, N, NLOC, NSLOT, NBLK):
    """Slot assignment + per-(block, src%4) padded edge segment arrays."""
    src = np.concatenate([E[0].astype(np.int64), np.arange(N, dtype=np.int64)])
    dst = np.concatenate([E[1].astype(np.int64), np.arange(N, dtype=np.int64)])
    deg = np.bincount(dst, minlength=N)

    # snake assignment over degree-sorted nodes -> balanced block edge loads
    order = np.argsort(-deg, kind="stable")
    r = np.arange(NSLOT)
    rnd, pos = divmod(r, NBLK)
    blk = np.where(rnd % 2 == 0, pos, NBLK - 1 - pos)
    slot_of_rank = blk * PT + rnd
    slot_of_node = np.empty(N, np.int64)
    slot_of_node[order] = slot_of_rank[:N]
    empties = slot_of_rank[N:]

    # keepalive self-edges for empty slots (their table rows are all-zero
    # with denom col 1 -> ex=1, denom=1, out=0)
    sslot = np.concatenate([slot_of_node[src], empties])
    dslot = np.concatenate([slot_of_node[dst], empties])

    key = (dslot >> 7) * 4 + (sslot & 3)
    order_e = np.argsort(key, kind="stable")
    ss = sslot[order_e]
    dd = dslot[order_e]
    kk = key[order_e]
    cnt = np.bincount(kk, minlength=NBLK * 4)
    T_seg = int(-(-cnt.max() // PT))
    SEG = T_seg * PT
    T_tot = 4 * T_seg

    seg_start = np.zeros(NBLK * 4 + 1, np.int64)
    np.cumsum(cnt, out=seg_start[1:])
    pos_e = np.arange(len(ss)) - seg_start[kk]
    dest = kk * SEG + pos_e

    tot = NBLK * 4 * SEG
    idx_src = np.zeros(tot, np.int16)          # gather row in [NSLOT/4, 4*ROW] view
    idx_dst = np.zeros(tot, np.int16)          # gather row in block's 128-row window
    dloc = np.full(tot, 128, np.uint8)         # 128 = pad sentinel (one-hot miss)
    idx_src[dest] = (ss >> 2).astype(np.int16)
    idx_dst[dest] = (dd & 127).astype(np.int16)
    dloc[dest] = (dd & 127).astype(np.uint8)

    NB = NBLK // NCORE
    # 16-partition wrap per gather list: idx j -> [j%16, j//16]
    a = idx_src.reshape(NCORE, NB, 4, T_seg * 8, 16)
    idxS = np.ascontiguousarray(a.transpose(0, 4, 1, 2, 3)).reshape(
        NCORE * 16, NB * 4 * T_seg * 8)
    b = idx_dst.reshape(NCORE, NB, T_tot * 8, 16)
    idxD = np.ascontiguousarray(b.transpose(0, 3, 1, 2)).reshape(
        NCORE * 16, NB * T_tot * 8)
    c = dloc.reshape(NCORE, NB, T_tot, PT)
    dloc8 = np.ascontiguousarray(c.transpose(0, 3, 1, 2)).reshape(
        NCORE * PT, NB * T_tot)
    return slot_of_node, idxS, idxD, dloc8, T_seg


def kernel(X, E, W1, att_src1, att_dst1, b1, W2, att_src2, att_dst2, b2):
    X = np.asarray(X, np.float32)
    E = np.asarray(E)
    W1 = np.asarray(W1, np.float32)
    W2 = np.asarray(W2, np.float32)
    as1 = np.asarray(att_src1, np.float32)
    ad1 = np.asarray(att_dst1, np.float32)
    as2 = np.asarray(att_src2, np.float32)
    ad2 = np.asarray(att_dst2, np.float32)
    b1 = np.asarray(b1, np.float32)
    b2 = np.asarray(b2, np.float32)

    N, F = X.shape                       # 100000, 256
    H, C = as1.shape                     # 2, 64
    C2 = as2.shape[1]                    # 64
    NLOC = -(-N // (NCORE * PT)) * PT    # 12544
    NSLOT = NLOC * NCORE                 # 100352
    NBLK = NSLOT // PT                   # 784
    NB = NBLK // NCORE                   # 98
    ROW1 = 192                           # [f0(64)|1|f1(64)|1|as0|as1|ad0|ad1|0*58]
    ROW2 = 128                           # [f(64)|1|as2|ad2|0*61]

    slot_of_node, idxS, idxD, dloc8, T_seg = _host_prep(X, E, N, NLOC, NSLOT, NBLK)
    SEG = T_seg * PT
    T_tot = 4 * T_seg
    colsS = NB * 4 * T_seg * 8
    colsD = NB * T_tot * 8

    # ---- weights in table-row layout
    W1e = np.zeros((F, ROW1), np.float32)
    W1e[:, 0:C] = W1[:, 0:C]
    W1e[:, C + 1:2 * C + 1] = W1[:, C:2 * C]
    W1e[:, 130] = W1[:, 0:C] @ as1[0]
    W1e[:, 131] = W1[:, C:2 * C] @ as1[1]
    W1e[:, 132] = W1[:, 0:C] @ ad1[0]
    W1e[:, 133] = W1[:, C:2 * C] @ ad1[1]

    W2e = np.zeros((H * C, ROW2), np.float32)
    W2e[:, 0:C2] = W2
    W2e[:, C2 + 1] = W2 @ as2[0]
    W2e[:, C2 + 2] = W2 @ ad2[0]

    Xs = np.zeros((NSLOT, F), np.float32)
    Xs[slot_of_node] = X
    XsT = np.ascontiguousarray(
        Xs.reshape(NCORE, NLOC, F).transpose(0, 2, 1)
    ).reshape(NCORE * F, NLOC).astype(ml_dtypes.bfloat16)

    W1e_bc = np.tile(W1e.astype(ml_dtypes.bfloat16), (NCORE, 1))
    W2e_bc = np.tile(W2e, (NCORE, 1))
    b1_bc = np.tile(b1[None, :].astype(np.float32), (NCORE * PT, 1))

    # ---------------- bass kernels ----------------
    @bass_jit
    def p1(nc, xsT, w1e):
        tab = nc.dram_tensor("tab", [NLOC, ROW1], F32, kind="ExternalOutput")
        nk = F // PT
        with tile.TileContext(nc) as tc:
            with (
                tc.tile_pool(name="wp", bufs=1) as wp,
                tc.tile_pool(name="xp", bufs=3) as xp,
                tc.tile_pool(name="pp", bufs=2, space="PSUM") as pp,
                tc.tile_pool(name="op", bufs=3) as op,
            ):
                wt = wp.tile([PT, nk, ROW1], BF16)
                for k in range(nk):
                    nc.sync.dma_start(wt[:, k, :], w1e[k * PT:(k + 1) * PT, :])
                for i in range(NLOC // PT):
                    xt = xp.tile([PT, nk, PT], BF16, tag="x", name=f"x{i}")
                    for k in range(nk):
                        nc.sync.dma_start(
                            xt[:, k, :], xsT[k * PT:(k + 1) * PT, i * PT:(i + 1) * PT])
                    ps = pp.tile([PT, ROW1], F32, tag="ps", name=f"ps{i}")
                    for k in range(nk):
                        nc.tensor.matmul(out=ps[:], lhsT=xt[:, k, :], rhs=wt[:, k, :],
                                         start=(k == 0), stop=(k == nk - 1))
                    og = op.tile([PT, ROW1], F32, tag="og", name=f"og{i}")
                    nc.scalar.activation(out=og[:], in_=ps[:], func=AF.Copy)
                    # denom const columns (psum there is exactly 0 -> +1)
                    nc.vector.tensor_scalar(out=og[:, C:C + 1], in0=og[:, C:C + 1],
                                            scalar1=1.0, scalar2=None, op0=OP.add)
                    nc.vector.tensor_scalar(out=og[:, 2 * C + 1:2 * C + 2],
                                            in0=og[:, 2 * C + 1:2 * C + 2],
                                            scalar1=1.0, scalar2=None, op0=OP.add)
                    nc.sync.dma_start(tab[i * PT:(i + 1) * PT, :], og[:])
        return tab

    import os
    _p2mode = int(os.environ.get("GAT_P2MODE", "0"))

    @bass_jit
    def p2(nc, g4, tab1, idxs, idxd, dl8, w2e, b1t):
        tab2 = nc.dram_tensor("tab2", [NLOC, ROW2], F32, kind="ExternalOutput")
        with tile.TileContext(nc) as tc:
            with (
                tc.tile_pool(name="st", bufs=1) as st,
                tc.tile_pool(name="hp", bufs=2) as hp,
                tc.tile_pool(name="hq", bufs=2) as hq,
                tc.tile_pool(name="eq", bufs=2) as eq,
                tc.tile_pool(name="sp", bufs=4) as sp,
                tc.tile_pool(name="pa", bufs=2, space="PSUM") as pa,
                tc.tile_pool(name="pb", bufs=2, space="PSUM") as pb,
                tc.tile_pool(name="ep", bufs=3) as ep,
            ):
                nc.gpsimd.load_library(mlp)
                ii = st.tile([PT, PT], I32)
                nc.gpsimd.iota(ii[:], pattern=[[1, PT]], base=0, channel_multiplier=0)
                iota_f = st.tile([PT, PT], F32)
                nc.vector.tensor_copy(iota_f[:], ii[:])
                ip = st.tile([PT, 1], I32)
                nc.gpsimd.iota(ip[:], pattern=[[1, 1]], base=0, channel_multiplier=1)
                ipf = st.tile([PT, 1], F32)
                nc.vector.tensor_copy(ipf[:], ip[:])
                ident = st.tile([PT, PT], F32)
                nc.vector.tensor_scalar(out=ident[:], in0=iota_f[:],
                                        scalar1=ipf[:, 0:1], scalar2=None,
                                        op0=OP.is_equal)
                isb = st.tile([PT, colsS], I16)
                idb = st.tile([PT, colsD], I16)
                for rr in range(8):
                    nc.sync.dma_start(isb[16 * rr:16 * (rr + 1), :], idxs[:, :])
                    nc.sync.dma_start(idb[16 * rr:16 * (rr + 1), :], idxd[:, :])
                d8 = st.tile([PT, NB * T_tot], U8)
                nc.sync.dma_start(d8[:], dl8[:, :])
                dlf = st.tile([PT, NB * T_tot], F32)
                nc.vector.tensor_copy(dlf[:], d8[:])
                wsb = st.tile([PT, ROW2], F32)
                nc.sync.dma_start(wsb[:], w2e[:, :])
                bsb = st.tile([PT, H * C], F32)
                nc.sync.dma_start(bsb[:], b1t[:, :])

                for b in range(NB):
                    hs = hp.tile([PT, T_tot, ROW1], F32, tag="hs", name=f"hs{b}")
                    for k in range(4):
                        nc.gpsimd.dma_gather(
                            hs[:, k * T_seg:(k + 1) * T_seg, :],
                            g4[:, k * ROW1:(k + 1) * ROW1],
                            isb[:, (b * 4 + k) * T_seg * 8:(b * 4 + k + 1) * T_seg * 8],
                            SEG, SEG, ROW1, elem_step=4 * ROW1, single_packet=False)
                    hd = hq.tile([PT, T_tot, C], F32, tag="hd", name=f"hd{b}")
                    nc.gpsimd.dma_gather(
                        hd[:], tab1[b * PT:(b + 1) * PT, 2 * C:3 * C],
                        idb[:, b * T_tot * 8:(b + 1) * T_tot * 8],
                        T_tot * PT, T_tot * PT, C, elem_step=ROW1, single_packet=False)
                    if _p2mode == 1:
                        h2x = ep.tile([PT, ROW2], F32, tag="h2", name=f"h2{b}")
                        nc.vector.tensor_tensor(out=h2x[:], in0=hs[:, 0, 0:ROW2],
                                                in1=hd[:, 0:2, :], op=OP.add)
                        nc.sync.dma_start(tab2[b * PT:(b + 1) * PT, :], h2x[:])
                        continue
                    exh = eq.tile([PT, H, T_tot], F32, tag="ex", name=f"ex{b}")
                    for h in range(H):
                        nc.vector.tensor_tensor(
                            out=exh[:, h, :], in0=hs[:, :, 130 + h:131 + h],
                            in1=hd[:, :, 4 + h:5 + h], op=OP.add)
                        nc.vector.scalar_tensor_tensor(
                            out=exh[:, h, :], in0=exh[:, h, :], scalar=NEG,
                            in1=exh[:, h, :], op0=OP.mult, op1=OP.max)
                        nc.scalar.activation(out=exh[:, h, :], in_=exh[:, h, :],
                                             func=AF.Exp)
                    if _p2mode == 2:
                        h2x = ep.tile([PT, ROW2], F32, tag="h2", name=f"h2{b}")
                        nc.vector.tensor_copy(h2x[:], hs[:, 0, 0:ROW2])
                        nc.vector.tensor_tensor(out=h2x[:, 0:T_tot],
                                                in0=h2x[:, 0:T_tot],
                                                in1=exh[:, 0, :], op=OP.add)
                        nc.sync.dma_start(tab2[b * PT:(b + 1) * PT, :], h2x[:])
                        continue
                    pss = [pa.tile([PT, C + 1], F32, tag=f"ps{h}", name=f"ps{b}_{h}")
                           for h in range(H)]
                    for t in range(T_tot):
                        for h in range(H):
                            S = sp.tile([PT, PT], F32, tag="S", name=f"S{b}_{t}_{h}")
                            nc.vector.tensor_scalar(
                                out=S[:], in0=iota_f[:],
                                scalar1=dlf[:, b * T_tot + t:b * T_tot + t + 1],
                                scalar2=exh[:, h, t:t + 1],
                                op0=OP.is_equal, op1=OP.mult)
                            nc.tensor.matmul(
                                out=pss[h][:], lhsT=S[:],
                                rhs=hs[:, t, h * (C + 1):(h + 1) * (C + 1)],
                                start=(t == 0), stop=(t == T_tot - 1))
                    r2 = ep.tile([PT, H], F32, tag="r", name=f"r{b}")
                    og = ep.tile([PT, H * C], F32, tag="og", name=f"og{b}")
                    for h in range(H):
                        nc.vector.reciprocal(r2[:, h:h + 1], pss[h][:, C:C + 1])
                        nc.vector.tensor_scalar(
                            out=og[:, h * C:(h + 1) * C], in0=pss[h][:, 0:C],
                            scalar1=r2[:, h:h + 1], scalar2=None, op0=OP.mult)
                    nc.vector.tensor_tensor(out=og[:], in0=og[:], in1=bsb[:],
                                            op=OP.add)
                    nc.vector.tensor_scalar(out=og[:], in0=og[:], scalar1=0.0,
                                            scalar2=None, op0=OP.max)
                    if _p2mode == 3:
                        h2x = ep.tile([PT, ROW2], F32, tag="h2", name=f"h2{b}")
                        nc.vector.tensor_copy(h2x[:], og[:])
                        nc.sync.dma_start(tab2[b * PT:(b + 1) * PT, :], h2x[:])
                        continue
                    # transpose via identity matmul: pt[f, s] = og[s, f]
                    pt = pb.tile([PT, PT], F32, tag="pt", name=f"pt{b}")
                    nc.tensor.matmul(out=pt[:], lhsT=og[:], rhs=ident[:],
                                     start=True, stop=True)
                    gt = ep.tile([PT, PT], F32, tag="gt", name=f"gt{b}")
                    nc.scalar.activation(out=gt[:], in_=pt[:], func=AF.Copy)
                    pm = pb.tile([PT, ROW2], F32, tag="pm", name=f"pm{b}")
                    nc.tensor.matmul(out=pm[:], lhsT=gt[:], rhs=wsb[:],
                                     start=True, stop=True)
                    h2 = ep.tile([PT, ROW2], F32, tag="h2", name=f"h2{b}")
                    nc.scalar.activation(out=h2[:], in_=pm[:], func=AF.Copy)
                    nc.vector.tensor_scalar(out=h2[:, C2:C2 + 1],
                                            in0=h2[:, C2:C2 + 1],
                                            scalar1=1.0, scalar2=None, op0=OP.add)
                    nc.sync.dma_start(tab2[b * PT:(b + 1) * PT, :], h2[:])
        return tab2

    @bass_jit
    def p3(nc, g24, tab2, idxs, idxd, dl8):
        outt = nc.dram_tensor("outp", [NLOC, C2], BF16, kind="ExternalOutput")
        with tile.TileContext(nc) as tc:
            with (
                tc.tile_pool(name="st", bufs=1) as st,
                tc.tile_pool(name="hp", bufs=2) as hp,
                tc.tile_pool(name="hq", bufs=2) as hq,
                tc.tile_pool(name="eq", bufs=2) as eq,
                tc.tile_pool(name="sp", bufs=4) as sp,
                tc.tile_pool(name="pa", bufs=4, space="PSUM") as pa,
                tc.tile_pool(name="ep", bufs=3) as ep,
            ):
                nc.gpsimd.load_library(mlp)
                ii = st.tile([PT, PT], I32)
                nc.gpsimd.iota(ii[:], pattern=[[1, PT]], base=0, channel_multiplier=0)
                iota_f = st.tile([PT, PT], F32)
                nc.vector.tensor_copy(iota_f[:], ii[:])
                isb = st.tile([PT, colsS], I16)
                idb = st.tile([PT, colsD], I16)
                for rr in range(8):
                    nc.sync.dma_start(isb[16 * rr:16 * (rr + 1), :], idxs[:, :])
                    nc.sync.dma_start(idb[16 * rr:16 * (rr + 1), :], idxd[:, :])
                d8 = st.tile([PT, NB * T_tot], U8)
                nc.sync.dma_start(d8[:], dl8[:, :])
                dlf = st.tile([PT, NB * T_tot], F32)
                nc.vector.tensor_copy(dlf[:], d8[:])

                for b in range(NB):
                    hs = hp.tile([PT, T_tot, ROW2], F32, tag="hs", name=f"hs{b}")
                    for k in range(4):
                        nc.gpsimd.dma_gather(
                            hs[:, k * T_seg:(k + 1) * T_seg, :],
                            g24[:, k * ROW2:(k + 1) * ROW2],
                            isb[:, (b * 4 + k) * T_seg * 8:(b * 4 + k + 1) * T_seg * 8],
                            SEG, SEG, ROW2, elem_step=4 * ROW2, single_packet=False)
                    hd = hq.tile([PT, T_tot, C2], F32, tag="hd", name=f"hd{b}")
                    nc.gpsimd.dma_gather(
                        hd[:], tab2[b * PT:(b + 1) * PT, C2:2 * C2],
                        idb[:, b * T_tot * 8:(b + 1) * T_tot * 8],
                        T_tot * PT, T_tot * PT, C2, elem_step=ROW2, single_packet=False)
                    exh = eq.tile([PT, T_tot], F32, tag="ex", name=f"ex{b}")
                    nc.vector.tensor_tensor(
                        out=exh[:], in0=hs[:, :, C2 + 1:C2 + 2],
                        in1=hd[:, :, 2:3], op=OP.add)
                    nc.vector.scalar_tensor_tensor(
                        out=exh[:], in0=exh[:], scalar=NEG, in1=exh[:],
                        op0=OP.mult, op1=OP.max)
                    nc.scalar.activation(out=exh[:], in_=exh[:], func=AF.Exp)
                    ps = pa.tile([PT, C2 + 1], F32, tag="ps", name=f"ps{b}")
                    for t in range(T_tot):
                        S = sp.tile([PT, PT], F32, tag="S", name=f"S{b}_{t}")
                        nc.vector.tensor_scalar(
                            out=S[:], in0=iota_f[:],
                            scalar1=dlf[:, b * T_tot + t:b * T_tot + t + 1],
                            scalar2=exh[:, t:t + 1],
                            op0=OP.is_equal, op1=OP.mult)
                        nc.tensor.matmul(out=ps[:], lhsT=S[:],
                                         rhs=hs[:, t, 0:C2 + 1],
                                         start=(t == 0), stop=(t == T_tot - 1))
                    r1 = ep.tile([PT, 1], F32, tag="r", name=f"r{b}")
                    nc.vector.reciprocal(r1[:, 0:1], ps[:, C2:C2 + 1])
                    ot = ep.tile([PT, C2], BF16, tag="ot", name=f"ot{b}")
                    nc.scalar.activation(out=ot[:], in_=ps[:, 0:C2], func=AF.Copy,
                                         scale=r1[:, 0:1])
                    nc.sync.dma_start(outt[b * PT:(b + 1) * PT, :], ot[:])
        return outt

    # ---------------- dispatch ----------------
    import threading

    devs = jax.devices()[:NCORE]
    mesh = Mesh(np.asarray(devs), ("core",))
    sh = NamedSharding(mesh, P("core"))

    t0 = time.time()
    xsT_d = jax.device_put(XsT, sh)
    w1_d = jax.device_put(W1e_bc, sh)
    w2_d = jax.device_put(W2e_bc, sh)
    b1_d = jax.device_put(b1_bc, sh)
    idxS_d = jax.device_put(idxS, sh)
    idxD_d = jax.device_put(idxD, sh)
    dloc_d = jax.device_put(dloc8, sh)

    smap = partial(shard_map, mesh=mesh, check_rep=False)
    p1j = jax.jit(smap(lambda x, w: p1(x, w),
                       in_specs=(P("core"),) * 2, out_specs=P("core")))

    def _ag1(t):
        g = jax.lax.all_gather(t, "core", axis=0, tiled=True)
        return g.reshape(NSLOT // 4, 4 * ROW1)

    ag1j = jax.jit(smap(_ag1, in_specs=(P("core"),), out_specs=P("core")))
    p2j = jax.jit(smap(lambda g, t, i1, i2, dl, w, bb: p2(g, t, i1, i2, dl, w, bb),
                       in_specs=(P("core"),) * 7, out_specs=P("core")))

    def _ag2(t):
        g = jax.lax.all_gather(t, "core", axis=0, tiled=True)
        return g.reshape(NSLOT // 4, 4 * ROW2)

    ag2j = jax.jit(smap(_ag2, in_specs=(P("core"),), out_specs=P("core")))
    p3j = jax.jit(smap(lambda g, t, i1, i2, dl: p3(g, t, i1, i2, dl),
                       in_specs=(P("core"),) * 5, out_specs=P("core")))

    # AOT-compile each stage on background threads so bass tracing and
    # executable load overlap with the input uploads above.
    BF = ml_dtypes.bfloat16

    def _sds(shape, dt):
        return jax.ShapeDtypeStruct(shape, dt, sharding=sh)

    s_tab1 = _sds((NCORE * NLOC, ROW1), np.float32)
    s_g4 = _sds((NCORE * NSLOT // 4, 4 * ROW1), np.float32)
    s_tab2 = _sds((NCORE * NLOC, ROW2), np.float32)
    s_g24 = _sds((NCORE * NSLOT // 4, 4 * ROW2), np.float32)
    specs = {
        "p1": (p1j, (_sds(XsT.shape, BF), _sds(W1e_bc.shape, BF))),
        "ag1": (ag1j, (s_tab1,)),
        "p2": (p2j, (s_g4, s_tab1, _sds(idxS.shape, np.int16),
                     _sds(idxD.shape, np.int16), _sds(dloc8.shape, np.uint8),
                     _sds(W2e_bc.shape, np.float32), _sds(b1_bc.shape, np.float32))),
        "ag2": (ag2j, (s_tab2,)),
        "p3": (p3j, (s_g24, s_tab2, _sds(idxS.shape, np.int16),
                     _sds(idxD.shape, np.int16), _sds(dloc8.shape, np.uint8))),
    }
    compiled = {}
    errs = {}

    def _compile_all():
        # serial on one thread: python tracing overlaps the uploads' network
        # waits; XLA/NEFF compile hits the persistent caches when warm.
        for name in ("p1", "ag1", "p2", "ag2", "p3"):
            try:
                f, sds_args = specs[name]
                compiled[name] = f.lower(*sds_args).compile()
            except Exception as e:  # fall back to plain jit call
                errs[name] = e

    th = threading.Thread(target=_compile_all)
    th.start()
    th.join()
    if errs:
        print(f"[gat] AOT compile fallback: {list(errs)} ({next(iter(errs.values()))!r})",
              flush=True)
    p1c = compiled.get("p1", p1j)
    ag1c = compiled.get("ag1", ag1j)
    p2c = compiled.get("p2", p2j)
    ag2c = compiled.get("ag2", ag2j)
    p3c = compiled.get("p3", p3j)

    import os
    _dbg = bool(int(os.environ.get("GAT_DEBUG", "0")))
    _tim = bool(int(os.environ.get("GAT_TIMING", "0")))

    def _ck(name, v):
        if _tim:
            jax.block_until_ready(v)
            t = time.time()
            print(f"[tim] {name}: +{t - _ck.t0:.3f}s", flush=True)
            _ck.t0 = t
        if _dbg:
            a = np.asarray(v)
            print(f"[dbg] {name}: shape={a.shape} dtype={a.dtype} "
                  f"finite={np.isfinite(a.astype(np.float32)).all()} "
                  f"absmax={np.abs(a.astype(np.float32)).max():.4g}", flush=True)
            DBG[name] = a
        return v

    _ck.t0 = t0
    _ck("compile+uploads", (xsT_d, w1_d, w2_d, b1_d, idxS_d, idxD_d, dloc_d))
    tab1 = _ck("tab1", p1c(xsT_d, w1_d))
    g4 = _ck("g4", ag1c(tab1))
    tab2 = _ck("tab2", p2c(g4, tab1, idxS_d, idxD_d, dloc_d, w2_d, b1_d))
    g24 = _ck("g24", ag2c(tab2))
    outg = _ck("p3", p3c(g24, tab2, idxS_d, idxD_d, dloc_d))
    out_slots = np.asarray(outg)
    if _tim:
        print(f"[tim] fetch: +{time.time() - _ck.t0:.3f}s", flush=True)
    LAST_WALL["ALL"] = time.time() - t0
    LAST_EXEC_NS["ALL"] = int(LAST_WALL["ALL"] * 1e9)

    res = out_slots.astype(np.float32)[slot_of_node]
    if np.any(b2):
        res = res + b2[None, :]
    return np.ascontiguousarray(res)


# revision 19
# speedup vs baseline: 2.3941x; 2.3941x over previous
"""GAT 2-layer kernel for Trainium2, 8 NeuronCores.

Strategy: nodes are permuted into 784 degree-balanced dst-blocks of 128
slots (98 blocks per core). All per-edge feature movement happens ON
DEVICE via gpsimd.dma_gather from a node-feature table in device HBM;
the halo exchange is an on-device jax.lax.all_gather between Bass
phases. Only ~60MB (bf16 X shards + compact int16 edge indices) crosses
the host<->device tunnel, vs ~1.5GB for host-side edge staging.

Pipeline (5 device dispatches, intermediates stay on device):
  p1 : tab1 = Xs @ W1e            -> per-core [NLOC, 192] f32 table rows
       row = [feat_h0(64) | 1 | feat_h1(64) | 1 | as0 as1 ad0 ad1 | 0pad]
  ag1: all_gather -> g4 [NSLOT/4, 768] (4 slots per row; int16 gather idx)
  p2 : layer-1 edge aggregation (dma_gather + one-hot-matmul scatter in
       PSUM) + relu + dense-2 -> tab2 [NLOC, 128] f32
       row = [feat(64) | 1 | as2 | ad2 | 0pad]
  ag2: all_gather -> g24 [NSLOT/4, 512]
  p3 : layer-2 edge aggregation -> out [NLOC, 64] bf16
"""
import time
from functools import partial

import numpy as np
import ml_dtypes

import jax

try:  # persistent XLA compile cache (saves ~8s/process on warm runs)
    jax.config.update("jax_compilation_cache_dir", "/tmp/gat_jax_cache")
    jax.config.update("jax_persistent_cache_min_compile_time_secs", 0.0)
    jax.config.update("jax_persistent_cache_min_entry_size_bytes", 0)
except Exception:
    pass

import jax.numpy as jnp
from jax.sharding import Mesh, NamedSharding, PartitionSpec as P
from jax.experimental.shard_map import shard_map

import concourse.bacc as bacc
import concourse.bass as bass
import concourse.mybir as mybir
import concourse.tile as tile
from concourse.bass2jax import bass_jit
from concourse.library_config import mlp

F32 = mybir.dt.float32
BF16 = mybir.dt.bfloat16
I16 = mybir.dt.int16
I32 = mybir.dt.int32
U8 = mybir.dt.uint8
AF = mybir.ActivationFunctionType
OP = mybir.AluOpType

PT = 128
NCORE = 8
NEG = 0.2

LAST_WALL = {}
LAST_EXEC_NS = {}
DBG = {}


def _host_prep(X, E, N, NLOC, NSLOT, NBLK):
    """Slot assignment + per-(block, src%4) padded edge segment arrays."""
    src = np.concatenate([E[0].astype(np.int64), np.arange(N, dtype=np.int64)])
    dst = np.concatenate([E[1].astype(np.int64), np.arange(N, dtype=np.int64)])
    deg = np.bincount(dst, minlength=N)

    # snake assignment over degree-sorted nodes -> balanced block edge loads
    order = np.argsort(-deg, kind="stable")
    r = np.arange(NSLOT)
    rnd, pos = divmod(r, NBLK)
    blk = np.where(rnd % 2 == 0, pos, NBLK - 1 - pos)
    slot_of_rank = blk * PT + rnd
    slot_of_node = np.empty(N, np.int64)
    slot_of_node[order] = slot_of_rank[:N]
    empties = slot_of_rank[N:]

    # keepalive self-edges for empty slots (their table rows are all-zero
    # with denom col 1 -> ex=1, denom=1, out=0)
    sslot = np.concatenate([slot_of_node[src], empties])
    dslot = np.concatenate([slot_of_node[dst], empties])

    key = (dslot >> 7) * 4 + (sslot & 3)
    order_e = np.argsort(key, kind="stable")
    ss = sslot[order_e]
    dd = dslot[order_e]
    kk = key[order_e]
    cnt = np.bincount(kk, minlength=NBLK * 4)
    T_seg = int(-(-cnt.max() // PT))
    SEG = T_seg * PT
    T_tot = 4 * T_seg

    seg_start = np.zeros(NBLK * 4 + 1, np.int64)
    np.cumsum(cnt, out=seg_start[1:])
    pos_e = np.arange(len(ss)) - seg_start[kk]
    dest = kk * SEG + pos_e

    tot = NBLK * 4 * SEG
    idx_src = np.zeros(tot, np.int16)          # gather row in [NSLOT/4, 4*ROW] view
    idx_dst = np.zeros(tot, np.int16)          # gather row in block's 128-row window
    dloc = np.full(tot, 128, np.uint8)         # 128 = pad sentinel (one-hot miss)
    idx_src[dest] = (ss >> 2).astype(np.int16)
    idx_dst[dest] = (dd & 127).astype(np.int16)
    dloc[dest] = (dd & 127).astype(np.uint8)

    NB = NBLK // NCORE
    # 16-partition wrap per gather list: idx j -> [j%16, j//16]
    a = idx_src.reshape(NCORE, NB, 4, T_seg * 8, 16)
    idxS = np.ascontiguousarray(a.transpose(0, 4, 1, 2, 3)).reshape(
        NCORE * 16, NB * 4 * T_seg * 8)
    b = idx_dst.reshape(NCORE, NB, T_tot * 8, 16)
    idxD = np.ascontiguousarray(b.transpose(0, 3, 1, 2)).reshape(
        NCORE * 16, NB * T_tot * 8)
    c = dloc.reshape(NCORE, NB, T_tot, PT)
    dloc8 = np.ascontiguousarray(c.transpose(0, 3, 1, 2)).reshape(
        NCORE * PT, NB * T_tot)
    return slot_of_node, idxS, idxD, dloc8, T_seg


def kernel(X, E, W1, att_src1, att_dst1, b1, W2, att_src2, att_dst2, b2):
    X = np.asarray(X, np.float32)
    E = np.asarray(E)
    W1 = np.asarray(W1, np.float32)
    W2 = np.asarray(W2, np.float32)
    as1 = np.asarray(att_src1, np.float32)
    ad1 = np.asarray(att_dst1, np.float32)
    as2 = np.asarray(att_src2, np.float32)
    ad2 = np.asarray(att_dst2, np.float32)
    b1 = np.asarray(b1, np.float32)
    b2 = np.asarray(b2, np.float32)

    N, F = X.shape                       # 100000, 256
    H, C = as1.shape                     # 2, 64
    C2 = as2.shape[1]                    # 64
    NLOC = -(-N // (NCORE * PT)) * PT    # 12544
    NSLOT = NLOC * NCORE                 # 100352
    NBLK = NSLOT // PT                   # 784
    NB = NBLK // NCORE                   # 98
    ROW1 = 192                           # [f0(64)|1|f1(64)|1|as0|as1|ad0|ad1|0*58]
    ROW2 = 128                           # [f(64)|1|as2|ad2|0*61]

    slot_of_node, idxS, idxD, dloc8, T_seg = _host_prep(X, E, N, NLOC, NSLOT, NBLK)
    SEG = T_seg * PT
    T_tot = 4 * T_seg
    colsS = NB * 4 * T_seg * 8
    colsD = NB * T_tot * 8

    # ---- weights in table-row layout
    W1e = np.zeros((F, ROW1), np.float32)
    W1e[:, 0:C] = W1[:, 0:C]
    W1e[:, C + 1:2 * C + 1] = W1[:, C:2 * C]
    W1e[:, 130] = W1[:, 0:C] @ as1[0]
    W1e[:, 131] = W1[:, C:2 * C] @ as1[1]
    W1e[:, 132] = W1[:, 0:C] @ ad1[0]
    W1e[:, 133] = W1[:, C:2 * C] @ ad1[1]

    W2e = np.zeros((H * C, ROW2), np.float32)
    W2e[:, 0:C2] = W2
    W2e[:, C2 + 1] = W2 @ as2[0]
    W2e[:, C2 + 2] = W2 @ ad2[0]

    Xs = np.zeros((NSLOT, F), np.float32)
    Xs[slot_of_node] = X
    XsT = np.ascontiguousarray(
        Xs.reshape(NCORE, NLOC, F).transpose(0, 2, 1)
    ).reshape(NCORE * F, NLOC).astype(ml_dtypes.bfloat16)

    W1e_bc = np.tile(W1e.astype(ml_dtypes.bfloat16), (NCORE, 1))
    W2e_bc = np.tile(W2e, (NCORE, 1))
    b1_bc = np.tile(b1[None, :].astype(np.float32), (NCORE * PT, 1))

    # ---------------- bass kernels ----------------
    @bass_jit
    def p1(nc, xsT, w1e):
        tab = nc.dram_tensor("tab", [NLOC, ROW1], F32, kind="ExternalOutput")
        nk = F // PT
        with tile.TileContext(nc) as tc:
            with (
                tc.tile_pool(name="wp", bufs=1) as wp,
                tc.tile_pool(name="xp", bufs=3) as xp,
                tc.tile_pool(name="pp", bufs=2, space="PSUM") as pp,
                tc.tile_pool(name="op", bufs=3) as op,
            ):
                wt = wp.tile([PT, nk, ROW1], BF16)
                for k in range(nk):
                    nc.sync.dma_start(wt[:, k, :], w1e[k * PT:(k + 1) * PT, :])
                for i in range(NLOC // PT):
                    xt = xp.tile([PT, nk, PT], BF16, tag="x", name=f"x{i}")
                    for k in range(nk):
                        nc.sync.dma_start(
                            xt[:, k, :], xsT[k * PT:(k + 1) * PT, i * PT:(i + 1) * PT])
                    ps = pp.tile([PT, ROW1], F32, tag="ps", name=f"ps{i}")
                    for k in range(nk):
                        nc.tensor.matmul(out=ps[:], lhsT=xt[:, k, :], rhs=wt[:, k, :],
                                         start=(k == 0), stop=(k == nk - 1))
                    og = op.tile([PT, ROW1], F32, tag="og", name=f"og{i}")
                    nc.scalar.activation(out=og[:], in_=ps[:], func=AF.Copy)
                    # denom const columns (psum there is exactly 0 -> +1)
                    nc.vector.tensor_scalar(out=og[:, C:C + 1], in0=og[:, C:C + 1],
                                            scalar1=1.0, scalar2=None, op0=OP.add)
                    nc.vector.tensor_scalar(out=og[:, 2 * C + 1:2 * C + 2],
                                            in0=og[:, 2 * C + 1:2 * C + 2],
                                            scalar1=1.0, scalar2=None, op0=OP.add)
                    nc.sync.dma_start(tab[i * PT:(i + 1) * PT, :], og[:])
        return tab

    import os
    _p2mode = int(os.environ.get("GAT_P2MODE", "0"))

    @bass_jit
    def p2(nc, g4, tab1, idxs, idxd, dl8, w2e, b1t):
        tab2 = nc.dram_tensor("tab2", [NLOC, ROW2], F32, kind="ExternalOutput")
        with tile.TileContext(nc) as tc:
            with (
                tc.tile_pool(name="st", bufs=1) as st,
                tc.tile_pool(name="hp", bufs=2) as hp,
                tc.tile_pool(name="hq", bufs=2) as hq,
                tc.tile_pool(name="eq", bufs=2) as eq,
                tc.tile_pool(name="sp", bufs=4) as sp,
                tc.tile_pool(name="pa", bufs=2, space="PSUM") as pa,
                tc.tile_pool(name="pb", bufs=2, space="PSUM") as pb,
                tc.tile_pool(name="ep", bufs=3) as ep,
            ):
                nc.gpsimd.load_library(mlp)
                ii = st.tile([PT, PT], I32)
                nc.gpsimd.iota(ii[:], pattern=[[1, PT]], base=0, channel_multiplier=0)
                iota_f = st.tile([PT, PT], F32)
                nc.vector.tensor_copy(iota_f[:], ii[:])
                ip = st.tile([PT, 1], I32)
                nc.gpsimd.iota(ip[:], pattern=[[1, 1]], base=0, channel_multiplier=1)
                ipf = st.tile([PT, 1], F32)
                nc.vector.tensor_copy(ipf[:], ip[:])
                ident = st.tile([PT, PT], F32)
                nc.vector.tensor_scalar(out=ident[:], in0=iota_f[:],
                                        scalar1=ipf[:, 0:1], scalar2=None,
                                        op0=OP.is_equal)
                isb = st.tile([PT, colsS], I16)
                idb = st.tile([PT, colsD], I16)
                for rr in range(8):
                    nc.sync.dma_start(isb[16 * rr:16 * (rr + 1), :], idxs[:, :])
                    nc.sync.dma_start(idb[16 * rr:16 * (rr + 1), :], idxd[:, :])
                d8 = st.tile([PT, NB * T_tot], U8)
                nc.sync.dma_start(d8[:], dl8[:, :])
                dlf = st.tile([PT, NB * T_tot], F32)
                nc.vector.tensor_copy(dlf[:], d8[:])
                wsb = st.tile([PT, ROW2], F32)
                nc.sync.dma_start(wsb[:], w2e[:, :])
                bsb = st.tile([PT, H * C], F32)
                nc.sync.dma_start(bsb[:], b1t[:, :])

                for b in range(NB):
                    hs = hp.tile([PT, T_tot, ROW1], F32, tag="hs", name=f"hs{b}")
                    for k in range(4):
                        nc.gpsimd.dma_gather(
                            hs[:, k * T_seg:(k + 1) * T_seg, :],
                            g4[:, k * ROW1:(k + 1) * ROW1],
                            isb[:, (b * 4 + k) * T_seg * 8:(b * 4 + k + 1) * T_seg * 8],
                            SEG, SEG, ROW1, elem_step=4 * ROW1, single_packet=False)
                    hd = hq.tile([PT, T_tot, C], F32, tag="hd", name=f"hd{b}")
                    nc.gpsimd.dma_gather(
                        hd[:], tab1[b * PT:(b + 1) * PT, 2 * C:3 * C],
                        idb[:, b * T_tot * 8:(b + 1) * T_tot * 8],
                        T_tot * PT, T_tot * PT, C, elem_step=ROW1, single_packet=False)
                    if _p2mode == 1:
                        h2x = ep.tile([PT, ROW2], F32, tag="h2", name=f"h2{b}")
                        nc.vector.tensor_tensor(out=h2x[:], in0=hs[:, 0, 0:ROW2],
                                                in1=hd[:, 0:2, :], op=OP.add)
                        nc.sync.dma_start(tab2[b * PT:(b + 1) * PT, :], h2x[:])
                        continue
                    exh = eq.tile([PT, H, T_tot], F32, tag="ex", name=f"ex{b}")
                    for h in range(H):
                        nc.vector.tensor_tensor(
                            out=exh[:, h, :], in0=hs[:, :, 130 + h:131 + h],
                            in1=hd[:, :, 4 + h:5 + h], op=OP.add)
                        nc.vector.scalar_tensor_tensor(
                            out=exh[:, h, :], in0=exh[:, h, :], scalar=NEG,
                            in1=exh[:, h, :], op0=OP.mult, op1=OP.max)
                        nc.scalar.activation(out=exh[:, h, :], in_=exh[:, h, :],
                                             func=AF.Exp)
                    if _p2mode == 2:
                        h2x = ep.tile([PT, ROW2], F32, tag="h2", name=f"h2{b}")
                        nc.vector.tensor_copy(h2x[:], hs[:, 0, 0:ROW2])
                        nc.vector.tensor_tensor(out=h2x[:, 0:T_tot],
                                                in0=h2x[:, 0:T_tot],
                                                in1=exh[:, 0, :], op=OP.add)
                        nc.sync.dma_start(tab2[b * PT:(b + 1) * PT, :], h2x[:])
                        continue
                    pss = [pa.tile([PT, C + 1], F32, tag=f"ps{h}", name=f"ps{b}_{h}")
                           for h in range(H)]
                    for t in range(T_tot):
                        for h in range(H):
                            S = sp.tile([PT, PT], F32, tag="S", name=f"S{b}_{t}_{h}")
                            nc.vector.tensor_scalar(
                                out=S[:], in0=iota_f[:],
                                scalar1=dlf[:, b * T_tot + t:b * T_tot + t + 1],
                                scalar2=exh[:, h, t:t + 1],
                                op0=OP.is_equal, op1=OP.mult)
                            nc.tensor.matmul(
                                out=pss[h][:], lhsT=S[:],
                                rhs=hs[:, t, h * (C + 1):(h + 1) * (C + 1)],
                                start=(t == 0), stop=(t == T_tot - 1))
                    r2 = ep.tile([PT, H], F32, tag="r", name=f"r{b}")
                    og = ep.tile([PT, H * C], F32, tag="og", name=f"og{b}")
                    for h in range(H):
                        nc.vector.reciprocal(r2[:, h:h + 1], pss[h][:, C:C + 1])
                        nc.vector.tensor_scalar(
                            out=og[:, h * C:(h + 1) * C], in0=pss[h][:, 0:C],
                            scalar1=r2[:, h:h + 1], scalar2=None, op0=OP.mult)
                    nc.vector.tensor_tensor(out=og[:], in0=og[:], in1=bsb[:],
                                            op=OP.add)
                    nc.vector.tensor_scalar(out=og[:], in0=og[:], scalar1=0.0,
                                            scalar2=None, op0=OP.max)
                    if _p2mode == 3:
                        h2x = ep.tile([PT, ROW2], F32, tag="h2", name=f"h2{b}")
                        nc.vector.tensor_copy(h2x[:], og[:])
                        nc.sync.dma_start(tab2[b * PT:(b + 1) * PT, :], h2x[:])
                        continue
                    # transpose via identity matmul: pt[f, s] = og[s, f]
                    pt = pb.tile([PT, PT], F32, tag="pt", name=f"pt{b}")
                    nc.tensor.matmul(out=pt[:], lhsT=og[:], rhs=ident[:],
                                     start=True, stop=True)
                    gt = ep.tile([PT, PT], F32, tag="gt", name=f"gt{b}")
                    nc.scalar.activation(out=gt[:], in_=pt[:], func=AF.Copy)
                    pm = pb.tile([PT, ROW2], F32, tag="pm", name=f"pm{b}")
                    nc.tensor.matmul(out=pm[:], lhsT=gt[:], rhs=wsb[:],
                                     start=True, stop=True)
                    h2 = ep.tile([PT, ROW2], F32, tag="h2", name=f"h2{b}")
                    nc.scalar.activation(out=h2[:], in_=pm[:], func=AF.Copy)
                    nc.vector.tensor_scalar(out=h2[:, C2:C2 + 1],
                                            in0=h2[:, C2:C2 + 1],
                                            scalar1=1.0, scalar2=None, op0=OP.add)
                    nc.sync.dma_start(tab2[b * PT:(b + 1) * PT, :], h2[:])
        return tab2

    @bass_jit
    def p3(nc, g24, tab2, idxs, idxd, dl8):
        outt = nc.dram_tensor("outp", [NLOC, C2], BF16, kind="ExternalOutput")
        with tile.TileContext(nc) as tc:
            with (
                tc.tile_pool(name="st", bufs=1) as st,
                tc.tile_pool(name="hp", bufs=2) as hp,
                tc.tile_pool(name="hq", bufs=2) as hq,
                tc.tile_pool(name="eq", bufs=2) as eq,
                tc.tile_pool(name="sp", bufs=4) as sp,
                tc.tile_pool(name="pa", bufs=4, space="PSUM") as pa,
                tc.tile_pool(name="ep", bufs=3) as ep,
            ):
                nc.gpsimd.load_library(mlp)
                ii = st.tile([PT, PT], I32)
                nc.gpsimd.iota(ii[:], pattern=[[1, PT]], base=0, channel_multiplier=0)
                iota_f = st.tile([PT, PT], F32)
                nc.vector.tensor_copy(iota_f[:], ii[:])
                isb = st.tile([PT, colsS], I16)
                idb = st.tile([PT, colsD], I16)
                for rr in range(8):
                    nc.sync.dma_start(isb[16 * rr:16 * (rr + 1), :], idxs[:, :])
                    nc.sync.dma_start(idb[16 * rr:16 * (rr + 1), :], idxd[:, :])
                d8 = st.tile([PT, NB * T_tot], U8)
                nc.sync.dma_start(d8[:], dl8[:, :])
                dlf = st.tile([PT, NB * T_tot], F32)
                nc.vector.tensor_copy(dlf[:], d8[:])

                for b in range(NB):
                    hs = hp.tile([PT, T_tot, ROW2], F32, tag="hs", name=f"hs{b}")
                    for k in range(4):
                        nc.gpsimd.dma_gather(
                            hs[:, k * T_seg:(k + 1) * T_seg, :],
                            g24[:, k * ROW2:(k + 1) * ROW2],
                            isb[:, (b * 4 + k) * T_seg * 8:(b * 4 + k + 1) * T_seg * 8],
                            SEG, SEG, ROW2, elem_step=4 * ROW2, single_packet=False)
                    hd = hq.tile([PT, T_tot, C2], F32, tag="hd", name=f"hd{b}")
                    nc.gpsimd.dma_gather(
                        hd[:], tab2[b * PT:(b + 1) * PT, C2:2 * C2],
                        idb[:, b * T_tot * 8:(b + 1) * T_tot * 8],
                        T_tot * PT, T_tot * PT, C2, elem_step=ROW2, single_packet=False)
                    exh = eq.tile([PT, T_tot], F32, tag="ex", name=f"ex{b}")
                    nc.vector.tensor_tensor(
                        out=exh[:], in0=hs[:, :, C2 + 1:C2 + 2],
                        in1=hd[:, :, 2:3], op=OP.add)
                    nc.vector.scalar_tensor_tensor(
                        out=exh[:], in0=exh[:], scalar=NEG, in1=exh[:],
                        op0=OP.mult, op1=OP.max)
                    nc.scalar.activation(out=exh[:], in_=exh[:], func=AF.Exp)
                    ps = pa.tile([PT, C2 + 1], F32, tag="ps", name=f"ps{b}")
                    for t in range(T_tot):
                        S = sp.tile([PT, PT], F32, tag="S", name=f"S{b}_{t}")
                        nc.vector.tensor_scalar(
                            out=S[:], in0=iota_f[:],
                            scalar1=dlf[:, b * T_tot + t:b * T_tot + t + 1],
                            scalar2=exh[:, t:t + 1],
                            op0=OP.is_equal, op1=OP.mult)
                        nc.tensor.matmul(out=ps[:], lhsT=S[:],
                                         rhs=hs[:, t, 0:C2 + 1],
                                         start=(t == 0), stop=(t == T_tot - 1))
                    r1 = ep.tile([PT, 1], F32, tag="r", name=f"r{b}")
                    nc.vector.reciprocal(r1[:, 0:1], ps[:, C2:C2 + 1])
                    ot = ep.tile([PT, C2], BF16, tag="ot", name=f"ot{b}")
                    nc.scalar.activation(out=ot[:], in_=ps[:, 0:C2], func=AF.Copy,
                                         scale=r1[:, 0:1])
                    nc.sync.dma_start(outt[b * PT:(b + 1) * PT, :], ot[:])
        return outt

    # ---------------- dispatch ----------------
    import threading

    devs = jax.devices()[:NCORE]
    mesh = Mesh(np.asarray(devs), ("core",))
    sh = NamedSharding(mesh, P("core"))

    t0 = time.time()
    xsT_d = jax.device_put(XsT, sh)
    w1_d = jax.device_put(W1e_bc, sh)
    w2_d = jax.device_put(W2e_bc, sh)
    b1_d = jax.device_put(b1_bc, sh)
    idxS_d = jax.device_put(idxS, sh)
    idxD_d = jax.device_put(idxD, sh)
    dloc_d = jax.device_put(dloc8, sh)

    smap = partial(shard_map, mesh=mesh, check_rep=False)
    p1j = jax.jit(smap(lambda x, w: p1(x, w),
                       in_specs=(P("core"),) * 2, out_specs=P("core")))

    def _ag1(t):
        g = jax.lax.all_gather(t, "core", axis=0, tiled=True)
        return g.reshape(NSLOT // 4, 4 * ROW1)

    ag1j = jax.jit(smap(_ag1, in_specs=(P("core"),), out_specs=P("core")))
    p2j = jax.jit(smap(lambda g, t, i1, i2, dl, w, bb: p2(g, t, i1, i2, dl, w, bb),
                       in_specs=(P("core"),) * 7, out_specs=P("core")))

    def _ag2(t):
        g = jax.lax.all_gather(t, "core", axis=0, tiled=True)
        return g.reshape(NSLOT // 4, 4 * ROW2)

    ag2j = jax.jit(smap(_ag2, in_specs=(P("core"),), out_specs=P("core")))
    p3j = jax.jit(smap(lambda g, t, i1, i2, dl: p3(g, t, i1, i2, dl),
                       in_specs=(P("core"),) * 5, out_specs=P("core")))

    # AOT-compile each stage on background threads so bass tracing and
    # executable load overlap with the input uploads above.
    BF = ml_dtypes.bfloat16

    def _sds(shape, dt):
        return jax.ShapeDtypeStruct(shape, dt, sharding=sh)

    s_tab1 = _sds((NCORE * NLOC, ROW1), np.float32)
    s_g4 = _sds((NCORE * NSLOT // 4, 4 * ROW1), np.float32)
    s_tab2 = _sds((NCORE * NLOC, ROW2), np.float32)
    s_g24 = _sds((NCORE * NSLOT // 4, 4 * ROW2), np.float32)
    specs = {
        "p1": (p1j, (_sds(XsT.shape, BF), _sds(W1e_bc.shape, BF))),
        "ag1": (ag1j, (s_tab1,)),
        "p2": (p2j, (s_g4, s_tab1, _sds(idxS.shape, np.int16),
                     _sds(idxD.shape, np.int16), _sds(dloc8.shape, np.uint8),
                     _sds(W2e_bc.shape, np.float32), _sds(b1_bc.shape, np.float32))),
        "ag2": (ag2j, (s_tab2,)),
        "p3": (p3j, (s_g24, s_tab2, _sds(idxS.shape, np.int16),
                     _sds(idxD.shape, np.int16), _sds(dloc8.shape, np.uint8))),
    }
    compiled = {}
    errs = {}

    def _compile_all():
        # serial on one thread: python tracing overlaps the uploads' network
        # waits; XLA/NEFF compile hits the persistent caches when warm.
        for name in ("p1", "ag1", "p2", "ag2", "p3"):
            try:
                f, sds_args = specs[name]
                compiled[name] = f.lower(*sds_args).compile()
            except Exception as e:  # fall back to plain jit call
                errs[name] = e

    _tc0 = time.time()
    th = threading.Thread(target=_compile_all)
    th.start()
    th.join()
    _compile_s = time.time() - _tc0
    if errs:
        print(f"[gat] AOT compile fallback: {list(errs)} ({next(iter(errs.values()))!r})",
              flush=True)
    p1c = compiled.get("p1", p1j)
    ag1c = compiled.get("ag1", ag1j)
    p2c = compiled.get("p2", p2j)
    ag2c = compiled.get("ag2", ag2j)
    p3c = compiled.get("p3", p3j)

    import os
    _dbg = bool(int(os.environ.get("GAT_DEBUG", "0")))
    _tim = bool(int(os.environ.get("GAT_TIMING", "0")))

    def _ck(name, v):
        if _tim:
            jax.block_until_ready(v)
            t = time.time()
            print(f"[tim] {name}: +{t - _ck.t0:.3f}s", flush=True)
            _ck.t0 = t
        if _dbg:
            a = np.asarray(v)
            print(f"[dbg] {name}: shape={a.shape} dtype={a.dtype} "
                  f"finite={np.isfinite(a.astype(np.float32)).all()} "
                  f"absmax={np.abs(a.astype(np.float32)).max():.4g}", flush=True)
            DBG[name] = a
        return v

    _ck.t0 = t0
    if _tim:
        print(f"[tim] compile-thread: {_compile_s:.3f}s", flush=True)
    _ck("compile+uploads", (xsT_d, w1_d, w2_d, b1_d, idxS_d, idxD_d, dloc_d))
    tab1 = _ck("tab1", p1c(xsT_d, w1_d))
    g4 = _ck("g4", ag1c(tab1))
    tab2 = _ck("tab2", p2c(g4, tab1, idxS_d, idxD_d, dloc_d, w2_d, b1_d))
    g24 = _ck("g24", ag2c(tab2))
    outg = _ck("p3", p3c(g24, tab2, idxS_d, idxD_d, dloc_d))
    out_slots = np.asarray(outg)
    if _tim:
        print(f"[tim] fetch: +{time.time() - _ck.t0:.3f}s", flush=True)
    LAST_WALL["ALL"] = time.time() - t0
    LAST_EXEC_NS["ALL"] = int(LAST_WALL["ALL"] * 1e9)

    res = out_slots.astype(np.float32)[slot_of_node]
    if np.any(b2):
        res = res + b2[None, :]
    return np.ascontiguousarray(res)
